# revision 1
# baseline (speedup 1.0000x reference)
"""HQQLinearLoRA TRN2 kernel: out = x @ W + (x @ A) @ B * 1.0 + bias.

Sharding: data-parallel over tokens (B*S) across 8 NeuronCores; W/bias/lora
replicated. Per core: [M_CORE, D] @ [D, D] with LoRA rank-16 + bias folded
into one extra K=17 accumulation matmul per output tile.

PE runs float32r (rounded fp32, 1 cycle/row). Every PE-instruction input is
last-produced by DVE so each fused-weight-load matmul carries at most one
sync wait (hardware limit on 4-byte-dtype matmuls).
"""
import numpy as np
from contextlib import ExitStack

import concourse.bass as bass
import concourse.tile as tile
import concourse.mybir as mybir
from concourse import bacc
from concourse.bass_utils import run_bass_kernel_spmd
from concourse.masks import make_identity

P = 128
NCORES = 8

# full problem dims (hardcoded per task contract)
B_DIM, S_DIM, D_DIM, R_DIM = 4, 4096, 4096, 16


def build_nc(m_core, d, r, m_blocks, n_tile=512, f32r=True,
             xs_bufs=2, ws_bufs=8, wr_bufs=3, ot_bufs=4, aux_bufs=2,
             dve_transpose=False):
    """One-core program; same program runs SPMD on all cores."""
    KT = d // P
    NT = d // n_tile
    mm_dt = mybir.dt.float32r if f32r else mybir.dt.float32
    f32 = mybir.dt.float32

    nc = bacc.Bacc(target_bir_lowering=False)
    x = nc.declare_dram_parameter("x", [m_core, d], f32, isOutput=False)
    W = nc.declare_dram_parameter("W", [d, d], f32, isOutput=False)
    bias = nc.declare_dram_parameter("bias", [d], f32, isOutput=False)
    lora_A = nc.declare_dram_parameter("lora_A", [d, r], f32, isOutput=False)
    lora_B = nc.declare_dram_parameter("lora_B", [r, d], f32, isOutput=False)
    out = nc.declare_dram_parameter("out", [m_core, d], f32, isOutput=True)

    with tile.TileContext(nc) as tc, ExitStack() as ctx:
        const = ctx.enter_context(tc.tile_pool(name="const", bufs=1))
        xstage = ctx.enter_context(tc.tile_pool(name="xstage", bufs=xs_bufs))
        xtpool = ctx.enter_context(tc.tile_pool(name="xtpool", bufs=1))
        wstage = ctx.enter_context(tc.tile_pool(name="wstage", bufs=ws_bufs))
        wrpool = ctx.enter_context(tc.tile_pool(name="wrpool", bufs=wr_bufs))
        stg = ctx.enter_context(tc.tile_pool(name="stg", bufs=2))
        outstage = ctx.enter_context(tc.tile_pool(name="outstage", bufs=ot_bufs))
        psum_main = ctx.enter_context(
            tc.tile_pool(name="psum_main", bufs=max(m_blocks), space="PSUM"))
        psum_aux = ctx.enter_context(
            tc.tile_pool(name="psum_aux", bufs=aux_bufs, space="PSUM"))

        # identity for PE transpose (fp32 path; HW-validated)
        ident = const.tile([P, P], f32)
        make_identity(nc, ident)

        # lora_A rounded, per k-tile: [P, r]
        a_r = []
        for ki in range(KT):
            ast = stg.tile([P, r], f32, name="ast")
            nc.sync.dma_start(ast[:], lora_A[ki * P:(ki + 1) * P, :])
            ar = const.tile([P, r], mm_dt, name=f"ar{ki}")
            nc.vector.tensor_copy(ar[:], ast[:])
            a_r.append(ar)

        # lora_B and bias rounded (separate tiles: partition bases must be 0)
        b_r = const.tile([r, d], mm_dt)
        bias_r = const.tile([1, d], mm_dt)
        for ni in range(NT):
            sl = slice(ni * n_tile, (ni + 1) * n_tile)
            bst = stg.tile([r, n_tile], f32, name="bst")
            nc.sync.dma_start(bst[:], lora_B[:, sl])
            nc.vector.tensor_copy(b_r[:, sl], bst[:])
            bist = stg.tile([1, n_tile], f32, name="bist")
            nc.sync.dma_start(bist[:], bias[sl].unsqueeze(0))
            nc.vector.tensor_copy(bias_r[:, sl], bist[:])

        # P1T = (x@A)^T: [r, m_core]; ones row for bias outer product
        p1t = const.tile([r, m_core], mm_dt)
        ones_st = const.tile([1, m_core], f32)
        nc.vector.memset(ones_st[:], 1.0)
        ones_r = const.tile([1, m_core], mm_dt)
        nc.vector.tensor_copy(ones_r[:], ones_st[:])

        mb_max = max(m_blocks)
        mt0 = 0  # running m-tile offset
        for mb in m_blocks:
            xtb = xtpool.tile([P, KT, mb_max * P], mm_dt, name="xtblock")
            # ---- transpose phase: x[mt*P:(mt+1)*P, :] -> xtb[:, ki, mi*P:]
            for mi in range(mb):
                mt = mt0 + mi
                xs = xstage.tile([P, d], f32, name="xs")
                nc.gpsimd.dma_start(xs[:], x[mt * P:(mt + 1) * P, :])
                if dve_transpose:
                    # 32x32-block DVE transpose straight into xtb (f32r out).
                    # Block row j of the output comes from partition strip j
                    # of the input with free offsets swapped.
                    for ki in range(KT):
                        for j in range(P // 32):
                            nc.vector.transpose(
                                xtb[:, ki, mi * P:(mi + 1) * P].rearrange(
                                    "p (b f) -> p b f", f=32)[32 * j:32 * (j + 1)]
                                .transpose(0, 1),
                                xs[:, ki * P + 32 * j: ki * P + 32 * (j + 1)]
                                .rearrange("(b q) f -> q b f", q=32),
                            )
                else:
                    for ki in range(KT):
                        pst = psum_aux.tile([P, P], f32, name="aux")
                        nc.tensor.transpose(pst[:], xs[:, ki * P:(ki + 1) * P],
                                            ident[:])
                        # copyback rounds to f32r for the main matmuls
                        nc.vector.tensor_copy(xtb[:, ki, mi * P:(mi + 1) * P],
                                              pst[:])
            # ---- P1T chunks for this block (free dim 256..512 per chunk)
            done = 0
            while done < mb:
                cn = min(4, mb - done)
                cs = cn * P
                psl = psum_aux.tile([r, 512], f32, name="aux")
                for ki in range(KT):
                    nc.tensor.matmul(
                        psl[:, :cs], a_r[ki][:],
                        xtb[:, ki, done * P:done * P + cs],
                        start=(ki == 0), stop=(ki == KT - 1))
                nc.vector.tensor_copy(
                    p1t[:, (mt0 + done) * P:(mt0 + done) * P + cs], psl[:, :cs])
                done += cn
            # ---- GEMM phase: stream W once per block
            for ni in range(NT):
                nsl = slice(ni * n_tile, (ni + 1) * n_tile)
                pss = [psum_main.tile([P, n_tile], f32, name="mm") for _ in range(mb)]
                for ki in range(KT):
                    # W goes on HWDGE with wstage bufs=8: slot reuse distance
                    # is a multiple of the 8 HWDGE sem lanes, so the WAW dep
                    # is same-lane (FIFO) and the DMA carries only the DVE
                    # recycle wait.
                    ws = wstage.tile([P, n_tile], f32, name="ws")
                    nc.sync.dma_start(ws[:], W[ki * P:(ki + 1) * P, nsl])
                    wr = wrpool.tile([P, n_tile], mm_dt, name="wr")
                    nc.vector.tensor_copy(wr[:], ws[:])
                    for mi in range(mb):
                        nc.tensor.matmul(
                            pss[mi][:], xtb[:, ki, mi * P:(mi + 1) * P],
                            wr[:], start=(ki == 0), stop=False)
                for mi in range(mb):
                    mt = mt0 + mi
                    nc.tensor.matmul(
                        pss[mi][:], p1t[:, mt * P:(mt + 1) * P], b_r[:, nsl],
                        start=False, stop=False)
                    nc.tensor.matmul(
                        pss[mi][:], ones_r[:, mt * P:(mt + 1) * P], bias_r[:, nsl],
                        start=False, stop=True)
                    ot = outstage.tile([P, n_tile], f32, name="ot")
                    nc.vector.tensor_copy(ot[:], pss[mi][:])
                    nc.gpsimd.dma_start(out[mt * P:(mt + 1) * P, nsl], ot[:])
            mt0 += mb
    nc.compile()
    return nc


_CACHE = {}


def _get_nc(key, *args, **kw):
    if key not in _CACHE:
        _CACHE[key] = build_nc(*args, **kw)
    return _CACHE[key]


def kernel(x, W, bias, lora_A, lora_B, _trace=False):
    Bb, S, D = x.shape
    R = lora_A.shape[1]
    M = Bb * S
    m_core = M // NCORES
    m_blocks = [4, 4, 4, 4]
    nc = _get_nc(("full", m_core, D, R), m_core, D, R, m_blocks)

    xf = np.ascontiguousarray(x.reshape(M, D), dtype=np.float32)
    W = np.ascontiguousarray(W, dtype=np.float32)
    bias = np.ascontiguousarray(bias, dtype=np.float32)
    lora_A = np.ascontiguousarray(lora_A, dtype=np.float32)
    lora_B = np.ascontiguousarray(lora_B, dtype=np.float32)

    in_maps = []
    for c in range(NCORES):
        in_maps.append({
            "x": xf[c * m_core:(c + 1) * m_core],
            "W": W, "bias": bias, "lora_A": lora_A, "lora_B": lora_B,
        })
    res = run_bass_kernel_spmd(nc, in_maps, list(range(NCORES)), trace=_trace)
    outs = [res.results[c]["out"] for c in range(NCORES)]
    full = np.concatenate(outs, axis=0).reshape(Bb, S, D).astype(x.dtype)
    if _trace:
        return full, res
    return full

